# revision 1
# baseline (speedup 1.0000x reference)
"""Segment-mean pooling (segment_sum / counts) + Linear, on 8 TRN2 NeuronCores.

Strategy: segment-ownership sharding.  The host sorts rows by dst_idx and
routes each row to the core that owns its segment range (core i owns
segments [512*i, 512*(i+1))), so no collectives are needed; the host
concatenates the 8 output shards.

Per core, the segment sums are computed in [segment, hidden] layout
(segments on PSUM partitions) in two passes:

  Pass 1 (banded): the host packs the first C=16 rows of every segment
  into a dense band of 16-row slots (~98% full).  A 128-row chunk then
  covers exactly 8 consecutive segments, and its segment-sum is ONE
  TensorE matmul: stationary = a constant block-ones [128, 32] matrix,
  moving = the x rows [128, 256].  No per-row index handling at all.

  Pass 2 (one-hot tail): rows beyond slot 16 (~4% of rows) go through
  windowed one-hot matmuls: VectorE builds is_equal one-hots against an
  iota row (precomputed during pass 1), and each chunk's matmuls write
  narrow 32-aligned windows of the accumulators.  The window schedule is
  shared across cores (min/max over cores) so the SPMD graph is
  identical on every core.

Both band and overflow arrays are shipped pre-swizzled as [128, k, 256]
so every DMA is a fully linear copy.  PSUM accumulators are zero-opened
by rank-1 matmuls, so all data matmuls are pure accumulates in any
order.  Every PSUM tensor is padded to a full private 2 KiB bank, and
VectorE only reads a bank once all TensorE writes to it are complete
(PE-write + DVE-read on one bank is a fatal HW error).

Epilogue: scale rows by 1/(count+eps) (host bincount shipped as a
reciprocal table), PE-transpose pooled to [hidden, segment], apply the
Linear as out[s, j] = pooled_T[:, s].T @ W.T[h, j] with fused bias-add
(per-tile pipelined), and DMA the [512, 256] f32 shard.
"""

import os

import numpy as np

import concourse.bass as bass
import concourse.mybir as mybir
from concourse.bass_utils import run_bass_kernel_spmd

N_CORES = 8
S_TOTAL = 4096
S_PER = S_TOTAL // N_CORES  # 512 segments per core
H = 256
EPS = np.float32(1e-8)
PAD_IDX = 9999.0  # sentinel relative idx; never matches iota [0, wmax2)
C = 16  # band-A capacity (rows per segment); must divide 128
C2 = 8  # band-B capacity (rows 16..24 of a segment); must divide 128

GSZ = 8  # chunks per band DMA (1024 rows, 512 KB)
KB = S_PER * C // 128  # 64 band-A chunks
KB2 = S_PER * C2 // 128  # 32 band-B chunks
N_BAND_GROUPS = KB // GSZ  # 8
N_B2_GROUPS = KB2 // GSZ  # 4

_graph_cache: dict = {}

if os.environ.get("K_LDW"):
    try:
        import libneuronxla.libncc as _ncc

        _ncc.NEURON_CC_FLAGS = [
            f.replace("--enable-ldw-opt=false", "--enable-ldw-opt=true")
            for f in _ncc.NEURON_CC_FLAGS
        ]
        os.environ["AXON_NCC_FLAGS"] = os.environ.get("AXON_NCC_FLAGS", "").replace(
            "--enable-ldw-opt=false", "--enable-ldw-opt=true"
        )
    except Exception:
        pass


def _build(ov_chunks: int, ov_parts: tuple, wmax2: int) -> "bass.Bass":
    """ov_parts[oc] = tuple of 32-aligned window-part start segments."""
    f16 = mybir.dt.float16
    f32 = mybir.dt.float32
    ovk = max(ov_chunks, 1)

    nc = bass.Bass()

    xb_d = nc.declare_dram_parameter("xb", [128, KB, H], f16, isOutput=False)
    xb2_d = nc.declare_dram_parameter("xb2", [128, KB2, H], f16, isOutput=False)
    xov_d = nc.declare_dram_parameter("xov", [128, ovk, H], f16, isOutput=False)
    ovidx_d = nc.declare_dram_parameter("ovidx", [128, ovk], f32, isOutput=False)
    iota_d = nc.declare_dram_parameter("iota", [128, wmax2 + 256], f16, isOutput=False)
    ones_d = nc.declare_dram_parameter("ones32", [128, 6, 32], f16, isOutput=False)
    ident_d = nc.declare_dram_parameter("ident", [128, 128], f16, isOutput=False)
    wt_d = nc.declare_dram_parameter("wt", [H, H], f16, isOutput=False)
    invc_d = nc.declare_dram_parameter("invc", [128, 4], f32, isOutput=False)
    bb_d = nc.declare_dram_parameter("bb", [128, H], f32, isOutput=False)
    out_d = nc.declare_dram_parameter("out", [S_PER, H], f32, isOutput=True)

    from contextlib import ExitStack

    with ExitStack() as ctx:
        xbb = ctx.enter_context(nc.sbuf_tensor("xbb", [128, KB, H], f16))
        xbb2 = ctx.enter_context(nc.sbuf_tensor("xbb2", [128, KB2, H], f16))
        xov_sb = ctx.enter_context(nc.sbuf_tensor("xov_sb", [128, ovk, H], f16))
        oh2 = ctx.enter_context(nc.sbuf_tensor("oh2", [128, ovk, wmax2], f16))
        ovidx_sb = ctx.enter_context(nc.sbuf_tensor("ovidx_sb", [128, ovk], f32))
        iota_sb = ctx.enter_context(nc.sbuf_tensor("iota_sb", [128, wmax2 + 256], f16))
        ones_sb = ctx.enter_context(nc.sbuf_tensor("ones_sb", [128, 6, 32], f16))
        ident_sb = ctx.enter_context(nc.sbuf_tensor("ident_sb", [128, 128], f16))
        wt_sb = ctx.enter_context(nc.sbuf_tensor("wt_sb", [128, 2, H], f16))
        invc_sb = ctx.enter_context(nc.sbuf_tensor("invc_sb", [128, 4], f32))
        bb_sb = ctx.enter_context(nc.sbuf_tensor("bb_sb", [128, H], f32))
        pool_sb = ctx.enter_context(nc.sbuf_tensor("pool_sb", [128, 4, H], f16))
        sums2_sb = ctx.enter_context(nc.sbuf_tensor("sums2_sb", [128, 2, S_PER], f16))
        out_sb = ctx.enter_context(nc.sbuf_tensor("out_sb", [128, 4, H], f32))
        # every PSUM tensor padded to one full private 2 KiB bank
        ps_s = [
            ctx.enter_context(nc.psum_tensor(f"ps_s{t}", [128, 512], f32))
            for t in range(4)
        ]
        ps_t = [
            ctx.enter_context(nc.psum_tensor(f"ps_t{hb}", [128, 1024], f16))
            for hb in range(2)
        ]
        ps_x = ctx.enter_context(nc.psum_tensor("ps_x", [128, 512], f32))
        dma_sem = ctx.enter_context(nc.semaphore("dma_sem"))
        csem = {
            name: ctx.enter_context(nc.semaphore(f"csem_{name}"))
            for name in ("iota", "ovidx", "ones", "ident", "wt", "invc", "bb")
        }
        bsem = [
            ctx.enter_context(nc.semaphore(f"bsem{g}"))
            for g in range(N_BAND_GROUPS)
        ]
        b2sem = [
            ctx.enter_context(nc.semaphore(f"b2sem{g}"))
            for g in range(N_B2_GROUPS)
        ]
        xsem = ctx.enter_context(nc.semaphore("xsem"))
        b2last = ctx.enter_context(nc.semaphore("b2last"))
        cmp_sem = ctx.enter_context(nc.semaphore("cmp_sem"))
        mm_sem = ctx.enter_context(nc.semaphore("mm_sem"))
        cp_sem = ctx.enter_context(nc.semaphore("cp_sem"))
        tr_sem = ctx.enter_context(nc.semaphore("tr_sem"))
        cp2_sem = ctx.enter_context(nc.semaphore("cp2_sem"))
        mme_sem = ctx.enter_context(nc.semaphore("mme_sem"))
        oe_sem = ctx.enter_context(nc.semaphore("oe_sem"))
        block = ctx.enter_context(nc.Block())

        zlhs = iota_sb[0:1, 0:128]  # junk values; multiplied by zero rhs
        zrhs = iota_sb[0:1, wmax2 : wmax2 + 256]  # zeros

        @block.sync
        def _(sync):
            # late-needed consts on the sync ring
            sync.dma_start(out=ident_sb[:, :], in_=ident_d[:, :]).then_inc(
                csem["ident"], 16
            )
            sync.dma_start(
                out=wt_sb[:, :, :],
                in_=wt_d[:, :].rearrange("(t p) j -> p t j", p=128),
            ).then_inc(csem["wt"], 16)
            sync.dma_start(out=invc_sb[:, :], in_=invc_d[:, :]).then_inc(
                csem["invc"], 16
            )
            sync.dma_start(out=bb_sb[:, :], in_=bb_d[:, :]).then_inc(csem["bb"], 16)
            for st in range(4):
                sync.wait_ge(oe_sem, st + 1)
                sync.dma_start(
                    out=out_d[st * 128 : (st + 1) * 128, :], in_=out_sb[:, st, :]
                ).then_inc(dma_sem, 16)
            for name in ("ident", "wt", "invc", "bb"):
                sync.wait_ge(csem[name], 16)
            sync.wait_ge(dma_sem, 16 * 4)

        @block.scalar
        def _(scalar):
            # ALL input DMAs on one ring, in consumption order, one
            # semaphore per DMA: cumulative thresholds on a shared sem
            # can't tell WHICH transfer completed.
            scalar.dma_start(out=ones_sb[:, :, :], in_=ones_d[:, :, :]).then_inc(
                csem["ones"], 16
            )
            scalar.dma_start(out=iota_sb[:, :], in_=iota_d[:, :]).then_inc(
                csem["iota"], 16
            )
            scalar.dma_start(out=ovidx_sb[:, :], in_=ovidx_d[:, :]).then_inc(
                csem["ovidx"], 16
            )
            scalar.dma_start(out=xov_sb[:, :, :], in_=xov_d[:, :, :]).then_inc(
                xsem, 16
            )
            for g in range(N_BAND_GROUPS):
                scalar.dma_start(
                    out=xbb[:, GSZ * g : GSZ * (g + 1), :],
                    in_=xb_d[:, GSZ * g : GSZ * (g + 1), :],
                ).then_inc(bsem[g], 16)
            for g in range(N_B2_GROUPS - 1):
                scalar.dma_start(
                    out=xbb2[:, GSZ * g : GSZ * (g + 1), :],
                    in_=xb2_d[:, GSZ * g : GSZ * (g + 1), :],
                ).then_inc(b2sem[g], 16)
            gl = N_B2_GROUPS - 1
            scalar.dma_start(
                out=xbb2[:, GSZ * gl : GSZ * gl + 4, :],
                in_=xb2_d[:, GSZ * gl : GSZ * gl + 4, :],
            ).then_inc(b2sem[gl], 16)
            scalar.dma_start(
                out=xbb2[:, GSZ * gl + 4 : GSZ * (gl + 1), :],
                in_=xb2_d[:, GSZ * gl + 4 : GSZ * (gl + 1), :],
            ).then_inc(b2last, 16)
            for g in range(N_BAND_GROUPS):
                scalar.wait_ge(bsem[g], 16)
            for g in range(N_B2_GROUPS - 1):
                scalar.wait_ge(b2sem[g], 16)
            scalar.wait_ge(b2sem[N_B2_GROUPS - 1], 16)
            scalar.wait_ge(b2last, 16)
            scalar.wait_ge(xsem, 16)
            for name in ("ones", "iota", "ovidx"):
                scalar.wait_ge(csem[name], 16)

        @block.vector
        def _(vector):
            # pass-2 one-hots, precomputed while PE runs the band pass
            if ov_chunks:
                vector.wait_ge(csem["iota"], 16)
                vector.wait_ge(csem["ovidx"], 16)
                for oc in range(ov_chunks):
                    woc = 32 * len(ov_parts[oc])
                    vector.tensor_scalar(
                        out=oh2[:, oc, 0:woc],
                        in0=iota_sb[:, 0:woc],
                        scalar1=ovidx_sb[:, oc : oc + 1],
                        scalar2=None,
                        op0=mybir.AluOpType.is_equal,
                    ).then_inc(cmp_sem, 1)
            # epilogue
            vector.wait_ge(mm_sem, 1)  # all accumulation done
            for st in range(4):
                vector.tensor_copy(
                    out=pool_sb[:, st, :], in_=ps_s[st][:, 0:H]
                ).then_inc(cp_sem, 1)
            # ps_t banks are PE-owned until ALL transposes finish
            vector.wait_ge(tr_sem, 4)
            for st in range(4):
                vector.tensor_copy(
                    out=sums2_sb[:, 0, 128 * st : 128 * (st + 1)],
                    in_=ps_t[0][:, 128 * st : 128 * (st + 1)],
                )
                vector.tensor_copy(
                    out=sums2_sb[:, 1, 128 * st : 128 * (st + 1)],
                    in_=ps_t[1][:, 128 * st : 128 * (st + 1)],
                ).then_inc(cp2_sem, 1)
            vector.wait_ge(csem["invc"], 16)
            vector.wait_ge(csem["bb"], 16)
            for st in range(4):
                vector.wait_ge(mme_sem, st + 1)
                vector.scalar_tensor_tensor(
                    out=out_sb[:, st, :],
                    in0=ps_s[st][:, 0:H],
                    scalar=invc_sb[:, st : st + 1],
                    in1=bb_sb[:, :],
                    op0=mybir.AluOpType.mult,
                    op1=mybir.AluOpType.add,
                ).then_inc(oe_sem, 1)

        @block.tensor
        def _(tensor):
            tensor.wait_ge(csem["iota"], 16)
            tensor.wait_ge(csem["ones"], 16)
            # warm the PE clock (HAM) while the first data DMAs are in
            # flight: ~3.4us of sustained matmul activity moves the PE
            # from 1.2 GHz to 2.4 GHz for the whole band pass
            for _ in range(14):
                tensor.matmul(
                    ps_x[:, 0:256], ident_sb[:, :], iota_sb[:, 0:256],
                    start=True, stop=True, skip_group_check=True,
                )
            # zero-open all four accumulators
            for t in range(4):
                tensor.matmul(
                    ps_s[t][:, 0:H], zlhs, zrhs, start=True, stop=False,
                    skip_group_check=True,
                )
            # overflow one-hot pass first: runs while the bands stream in
            if ov_chunks:
                tensor.wait_ge(xsem, 16)
                for oc in range(ov_chunks):
                    tensor.wait_ge(cmp_sem, oc + 1)
                    for pi, seg0 in enumerate(ov_parts[oc]):
                        t, poff = seg0 // 128, seg0 % 128
                        tensor.matmul(
                            ps_s[t][poff : poff + 32, 0:H],
                            oh2[:, oc, 32 * pi : 32 * (pi + 1)],
                            xov_sb[:, oc, :],
                            start=False,
                            stop=False,
                            skip_group_check=True,
                            tile_position=(0, poff),
                        )
            # band A: chunk c covers segs [8c, 8c+8)
            for c in range(KB):
                if c % GSZ == 0:
                    tensor.wait_ge(bsem[c // GSZ], 16)
                    # full-width pulse so the HAM keeps the PE at 2.4 GHz
                    # (M=32 band matmuls alone do not register as busy)
                    tensor.matmul(
                        ps_x[:, 0:512], ident_sb[:, :],
                        xbb[:, GSZ * (c // GSZ) : GSZ * (c // GSZ) + 2, :],
                        start=True, stop=True, skip_group_check=True,
                    )
                v, j = divmod(c, 16)
                t, poff = j // 4, 32 * (j % 4)
                tensor.matmul(
                    ps_s[t][poff : poff + 32, 0:H],
                    ones_sb[:, v, :],
                    xbb[:, c, :],
                    start=False,
                    stop=False,
                    skip_group_check=True,
                    tile_position=(0, poff),
                )
            # band B: chunk c covers segs [16c, 16c+16)
            for c in range(KB2):
                g2 = c // GSZ
                if c % GSZ == 0:
                    tensor.wait_ge(b2sem[g2], 16)
                    tensor.matmul(
                        ps_x[:, 0:256], ident_sb[:, :], iota_sb[:, 0:256],
                        start=True, stop=True, skip_group_check=True,
                    )
                if g2 == N_B2_GROUPS - 1 and c % GSZ == 4:
                    tensor.wait_ge(b2last, 16)
                u, j = divmod(c, 16)
                t, poff = j // 4, 32 * (j % 4)
                tensor.matmul(
                    ps_s[t][poff : poff + 32, 0:H],
                    ones_sb[:, 4 + u, :],
                    xbb2[:, c, :],
                    start=False,
                    stop=False,
                    skip_group_check=True,
                    tile_position=(0, poff),
                )
            # close the accumulators
            for t in range(4):
                tensor.matmul(
                    ps_s[t][:, 0:H], zlhs, zrhs, start=False, stop=True,
                    skip_group_check=True,
                )
            # fence: matmul ends are FIFO; a matmul's then_inc can fire
            # before its PSUM writes drain, so hand banks to DVE only
            # after a trailing fence matmul completes
            tensor.matmul(
                ps_x[:, 0:H], zlhs, zrhs, start=True, stop=True,
                skip_group_check=True,
            ).then_inc(mm_sem, 1)
            # transposes: pooled [s, h] -> pooled_T [h, s], per tile
            tensor.wait_ge(csem["ident"], 16)
            for st in range(4):
                tensor.wait_ge(cp_sem, st + 1)
                for hb in range(2):
                    ins = tensor.transpose(
                        ps_t[hb][:, 128 * st : 128 * (st + 1)],
                        pool_sb[:, st, 128 * hb : 128 * (hb + 1)],
                        ident_sb[:, :],
                    )
                if st < 3:
                    ins.then_inc(tr_sem, 1)
                else:
                    tensor.matmul(
                        ps_x[:, 0:H], zlhs, zrhs, start=True, stop=True,
                        skip_group_check=True,
                    ).then_inc(tr_sem, 1)
            # Linear: out[s, j] = sum_h pooled_T[h, s] * wt[h, j]
            tensor.wait_ge(csem["wt"], 16)
            for st in range(4):
                tensor.wait_ge(cp2_sem, st + 1)
                tensor.matmul(
                    ps_s[st][:, 0:H],
                    sums2_sb[:, 0, st * 128 : (st + 1) * 128],
                    wt_sb[:, 0, :],
                    start=True,
                    stop=False,
                )
                tensor.matmul(
                    ps_s[st][:, 0:H],
                    sums2_sb[:, 1, st * 128 : (st + 1) * 128],
                    wt_sb[:, 1, :],
                    start=False,
                    stop=True,
                )
                tensor.matmul(
                    ps_x[:, 0:H], zlhs, zrhs, start=True, stop=True,
                    skip_group_check=True,
                ).then_inc(mme_sem, 1)

    return nc


def kernel(x, dst_idx, dst_size, W, b):
    x = np.asarray(x)
    idx = np.asarray(dst_idx).astype(np.int64)
    W = np.asarray(W, dtype=np.float32)
    b = np.asarray(b, dtype=np.float32)
    S = int(dst_size)
    assert S == S_TOTAL and x.shape[1] == H

    counts = np.bincount(idx, minlength=S).astype(np.float32)
    inv = np.float32(1.0) / (counts + EPS)  # [4096] f32

    order = np.argsort(idx, kind="stable")
    sidx = idx[order]
    bounds = np.searchsorted(sidx, np.arange(0, S + 1, S_PER))

    x16 = x.astype(np.float16)

    # split each core's rows into band A (rank < C), band B
    # (C <= rank < C+C2), and overflow (rank >= C+C2)
    bands, bands2, ovs, ovsegs = [], [], [], []
    for i in range(N_CORES):
        lo_i, hi_i = bounds[i], bounds[i + 1]
        n_i = hi_i - lo_i
        li = (sidx[lo_i:hi_i] - S_PER * i).astype(np.int64)
        rows = order[lo_i:hi_i]
        starts = np.searchsorted(li, np.arange(S_PER + 1))
        rank = np.arange(n_i) - starts[li]
        bm = rank < C
        sa = li[bm]
        slot = (16 * ((sa % 32) // 8) + sa // 32) * 128 + (sa % 8) * C + rank[bm]
        xband = np.zeros((128, KB, H), dtype=np.float16)
        xband[slot % 128, slot // 128] = x16[rows[bm]]
        bands.append(xband)
        bm2 = (rank >= C) & (rank < C + C2)
        sb = li[bm2]
        slot2 = (16 * ((sb % 32) // 16) + sb // 32) * 128 + (sb % 16) * C2 + (
            rank[bm2] - C
        )
        xband2 = np.zeros((128, KB2, H), dtype=np.float16)
        xband2[slot2 % 128, slot2 // 128] = x16[rows[bm2]]
        bands2.append(xband2)
        om = rank >= C + C2
        ovs.append(x16[rows[om]])
        ovsegs.append(li[om])

    ov_chunks = max(-(-len(s) // 128) for s in ovsegs)
    ovk = max(ov_chunks, 1)

    # shared overflow window schedule (32-aligned part starts)
    wins, parts = [], []
    for oc in range(ov_chunks):
        lo_w, hi_w = S_PER - 1, 0
        for s in ovsegs:
            seg = s[128 * oc : 128 * (oc + 1)]
            if len(seg):
                lo_w = min(lo_w, int(seg[0]))
                hi_w = max(hi_w, int(seg[-1]))
        hi_w = max(hi_w, lo_w)
        w = (lo_w // 32) * 32
        wins.append(w)
        parts.append(tuple(range(w, (hi_w // 32) * 32 + 32, 32)))
    wmax2 = max((len(p) for p in parts), default=1) * 32
    parts_t = tuple(parts)

    key = (ov_chunks, parts_t, wmax2)
    nc = _graph_cache.get(key)
    if nc is None:
        nc = _build(ov_chunks, parts_t, wmax2)
        _graph_cache[key] = nc

    iota_np = np.zeros((128, wmax2 + 256), dtype=np.float16)
    iota_np[:, :wmax2] = np.arange(wmax2, dtype=np.float16)
    ones_np = np.zeros((128, 6, 32), dtype=np.float16)
    r = np.arange(128)
    for v in range(4):
        ones_np[r, v, 8 * v + r // C] = 1.0
    for u in range(2):
        ones_np[r, 4 + u, 16 * u + r // C2] = 1.0
    ident_np = np.eye(128, dtype=np.float16)
    wt_np = np.ascontiguousarray(W.T).astype(np.float16)
    bb_np = np.ascontiguousarray(np.tile(b, (128, 1)), dtype=np.float32)

    in_maps = []
    for i in range(N_CORES):
        n_ov = len(ovsegs[i])
        xov = np.zeros((128, ovk, H), dtype=np.float16)
        ro = np.arange(n_ov)
        xov[ro % 128, ro // 128] = ovs[i]
        ovidx = np.full((128, ovk), PAD_IDX, dtype=np.float32)
        if ov_chunks:
            ovidx[ro % 128, ro // 128] = ovsegs[i] - np.repeat(wins, 128)[:n_ov]
        invc_np = np.ascontiguousarray(
            inv[S_PER * i : S_PER * (i + 1)].reshape(4, 128).T
        )
        in_maps.append(
            {
                "xb": bands[i],
                "xb2": bands2[i],
                "xov": xov,
                "ovidx": ovidx,
                "iota": iota_np,
                "ones32": ones_np,
                "ident": ident_np,
                "wt": wt_np,
                "invc": invc_np,
                "bb": bb_np,
            }
        )

    res = run_bass_kernel_spmd(nc, in_maps, core_ids=list(range(N_CORES)))
    return np.concatenate([res.results[i]["out"] for i in range(N_CORES)], axis=0)



# revision 4
# speedup vs baseline: 1.1022x; 1.1022x over previous
"""Segment-mean pooling (segment_sum / counts) + Linear, on 8 TRN2 NeuronCores.

Segment-ownership sharding: the host routes each row to the core that owns
its segment range (core i owns segments [512*i, 512*(i+1))); no collectives.

Per core, segments are split into 4 tiles of 128 (one PSUM bank each), and
the input stream is ordered TILE-MAJOR so each tile's epilogue (scale +
transpose + Linear + store) runs on otherwise-idle engines while the next
tile's rows are still streaming in.  Only the last tile's epilogue is on
the post-stream critical path.

Per tile the rows arrive in three forms:
  - band A: the first 16 rows of every segment, packed so that 4 chunks of
    128 rows form a quad of TensorE matmuls against 4 shared block-ones
    [128, 32] stationaries, one per 32-partition column group (the PE runs
    the 4 members concurrently).  The first quad opens the PSUM bank
    (start=True); everything later accumulates.
  - band B: rows 16..24, same quad structure with 8-row slots.
  - overflow: rows 24+ (~4%), 128-row chunks with a DVE-built is_equal
    one-hot [128, 128] stationary; the last chunk closes the bank
    (stop=True).

No HAM warmup / pulse matmuls: the kernel is sized for the 1.2 GHz cold PE
clock, at which quad throughput (~77 ns/chunk) still beats the ~350 GB/s
DMA delivery rate (~180 ns/chunk).

Epilogue per tile: fence matmul -> DVE cast f32->f16 -> PE transpose of the
two [128, 128] halves (alternating ps banks by tile parity) -> DVE copy to
SBUF -> two Linear matmuls out[s, j] = sum_h pooled_T[h, s] * W.T[h, j]
-> fence -> DVE scale_by_1/count + bias -> DMA the [128, 256] f32 shard.
"""

import numpy as np

import concourse.bass as bass
import concourse.mybir as mybir
from concourse.bass_utils import run_bass_kernel_spmd

N_CORES = 8
S_TOTAL = 4096
S_PER = S_TOTAL // N_CORES  # 512 segments per core
N_TILES = 4  # PSUM tiles of 128 segments
H = 256
EPS = np.float32(1e-8)
PAD_IDX = 9999.0  # sentinel relative idx; never matches iota [0, 128)
C = 16  # band-A capacity (rows per segment)
C2 = 8  # band-B capacity (rows 16..24)

KA = 16  # band-A chunks per tile
KB2 = 8  # band-B chunks per tile

# cf16 const layout (f16 columns)
ONES_OFF = 0  # 6 patterns x 32 (A g0..g3, B h0..h1)
IDENT_OFF = 192
IOTA_OFF = 320
WT_OFF = 448  # 2 x 256
CF16_W = 960
# cf32 const layout (f32 columns): invc[4], bb[256], ovidx[OVK]
BB_OFF = 4
OVIDX_OFF = 260

_graph_cache: dict = {}


def _build(ovks: tuple) -> "bass.Bass":
    """ovks[t] = number of overflow chunks for tile t (>=1, SPMD-shared)."""
    f16 = mybir.dt.float16
    f32 = mybir.dt.float32
    OVK = sum(ovks)
    KR = KB2 + max(ovks) * 0  # per-tile xrest chunks vary; total below
    NREST = 4 * KB2 + OVK
    roff = [0]
    for t in range(N_TILES):
        roff.append(roff[-1] + KB2 + ovks[t])

    nc = bass.Bass()

    xb_d = nc.declare_dram_parameter("xb", [128, 64, H], f16, isOutput=False)
    xr_d = nc.declare_dram_parameter("xr", [128, NREST, H], f16, isOutput=False)
    cf16_d = nc.declare_dram_parameter("cf16", [128, CF16_W], f16, isOutput=False)
    cf32_d = nc.declare_dram_parameter(
        "cf32", [128, OVIDX_OFF + OVK], f32, isOutput=False
    )
    out_d = nc.declare_dram_parameter("out", [S_PER, H], f32, isOutput=True)

    from contextlib import ExitStack

    with ExitStack() as ctx:
        xbb = ctx.enter_context(nc.sbuf_tensor("xbb", [128, 64, H], f16))
        xrr = ctx.enter_context(nc.sbuf_tensor("xrr", [128, NREST, H], f16))
        cf16 = ctx.enter_context(nc.sbuf_tensor("cf16s", [128, CF16_W], f16))
        cf32 = ctx.enter_context(
            nc.sbuf_tensor("cf32s", [128, OVIDX_OFF + OVK], f32)
        )
        oh = ctx.enter_context(nc.sbuf_tensor("oh", [128, OVK, 128], f16))
        pool = ctx.enter_context(nc.sbuf_tensor("pool", [128, N_TILES, H], f16))
        sums2 = ctx.enter_context(nc.sbuf_tensor("sums2", [128, 8, 128], f16))
        out_sb = ctx.enter_context(nc.sbuf_tensor("outsb", [128, N_TILES, H], f32))
        ps_s = [
            ctx.enter_context(nc.psum_tensor(f"ps_s{t}", [128, 512], f32))
            for t in range(N_TILES)
        ]
        ps_tab = [
            ctx.enter_context(nc.psum_tensor(f"ps_tab{i}", [128, 1024], f16))
            for i in range(2)
        ]
        ps_x = ctx.enter_context(nc.psum_tensor("ps_x", [128, 512], f32))

        a1sem = [ctx.enter_context(nc.semaphore(f"a1s{t}")) for t in range(4)]
        a2sem = [ctx.enter_context(nc.semaphore(f"a2s{t}")) for t in range(4)]
        rsem = [ctx.enter_context(nc.semaphore(f"rs{t}")) for t in range(4)]
        c16sem = ctx.enter_context(nc.semaphore("c16sem"))
        c32sem = ctx.enter_context(nc.semaphore("c32sem"))
        ohsem = ctx.enter_context(nc.semaphore("ohsem"))
        mmsem = ctx.enter_context(nc.semaphore("mmsem"))
        castsem = ctx.enter_context(nc.semaphore("castsem"))
        trsem = ctx.enter_context(nc.semaphore("trsem"))
        cp2sem = ctx.enter_context(nc.semaphore("cp2sem"))
        mmesem = ctx.enter_context(nc.semaphore("mmesem"))
        oesem = ctx.enter_context(nc.semaphore("oesem"))
        dmasem = ctx.enter_context(nc.semaphore("dmasem"))
        block = ctx.enter_context(nc.Block())

        ident = cf16[:, IDENT_OFF : IDENT_OFF + 128]
        iota = cf16[:, IOTA_OFF : IOTA_OFF + 128]
        zl = cf16[0:1, 0:128]  # junk; fence targets ps_x which is never read
        zr = cf16[0:1, 0:8]

        @block.scalar
        def _(scalar):
            # all x DMAs on one ring, tile-major, in consumption order
            for t in range(N_TILES):
                scalar.dma_start(
                    out=xbb[:, 16 * t : 16 * t + 8, :],
                    in_=xb_d[:, 16 * t : 16 * t + 8, :],
                ).then_inc(a1sem[t], 16)
                scalar.dma_start(
                    out=xbb[:, 16 * t + 8 : 16 * t + 16, :],
                    in_=xb_d[:, 16 * t + 8 : 16 * t + 16, :],
                ).then_inc(a2sem[t], 16)
                scalar.dma_start(
                    out=xrr[:, roff[t] : roff[t + 1], :],
                    in_=xr_d[:, roff[t] : roff[t + 1], :],
                ).then_inc(rsem[t], 16)
            for t in range(N_TILES):
                scalar.wait_ge(a1sem[t], 16)
                scalar.wait_ge(a2sem[t], 16)
                scalar.wait_ge(rsem[t], 16)

        @block.sync
        def _(sync):
            sync.dma_start(out=cf16[:, :], in_=cf16_d[:, :]).then_inc(c16sem, 16)
            sync.dma_start(out=cf32[:, :], in_=cf32_d[:, :]).then_inc(c32sem, 16)
            for t in range(N_TILES):
                sync.wait_ge(oesem, t + 1)
                sync.dma_start(
                    out=out_d[128 * t : 128 * (t + 1), :], in_=out_sb[:, t, :]
                ).then_inc(dmasem, 16)
            sync.wait_ge(c16sem, 16)
            sync.wait_ge(c32sem, 16)
            sync.wait_ge(dmasem, 16 * N_TILES)

        @block.vector
        def _(vector):
            # overflow one-hots, built up-front while the stream runs
            vector.wait_ge(c16sem, 16)
            vector.wait_ge(c32sem, 16)
            for j in range(OVK):
                vector.tensor_scalar(
                    out=oh[:, j, :],
                    in0=iota,
                    scalar1=cf32[:, OVIDX_OFF + j : OVIDX_OFF + j + 1],
                    scalar2=None,
                    op0=mybir.AluOpType.is_equal,
                ).then_inc(ohsem, 1)
            # per-tile epilogue stages
            for t in range(N_TILES):
                vector.wait_ge(mmsem, t + 1)
                vector.tensor_copy(out=pool[:, t, :], in_=ps_s[t][:, 0:H]).then_inc(
                    castsem, 1
                )
                vector.wait_ge(trsem, t + 1)
                vector.tensor_copy(
                    out=sums2[:, 2 * t, :], in_=ps_tab[t % 2][:, 0:128]
                )
                vector.tensor_copy(
                    out=sums2[:, 2 * t + 1, :], in_=ps_tab[t % 2][:, 128:256]
                ).then_inc(cp2sem, 1)
                vector.wait_ge(mmesem, t + 1)
                vector.scalar_tensor_tensor(
                    out=out_sb[:, t, :],
                    in0=ps_s[t][:, 0:H],
                    scalar=cf32[:, t : t + 1],
                    in1=cf32[:, BB_OFF : BB_OFF + H],
                    op0=mybir.AluOpType.mult,
                    op1=mybir.AluOpType.add,
                ).then_inc(oesem, 1)

        @block.tensor
        def _(tensor):
            tensor.wait_ge(c16sem, 16)
            ohcum = 0
            for t in range(N_TILES):
                # band A: 4 quads; quad g=0 opens the bank
                tensor.wait_ge(a1sem[t], 16)
                for g in range(2):
                    for m in range(4):
                        tensor.matmul(
                            ps_s[t][32 * m : 32 * m + 32, 0:H],
                            cf16[:, 32 * g : 32 * g + 32],
                            xbb[:, 16 * t + 4 * g + m, :],
                            start=(g == 0),
                            stop=False,
                            skip_group_check=True,
                            tile_position=(0, 32 * m),
                        )
                tensor.wait_ge(a2sem[t], 16)
                for g in range(2, 4):
                    for m in range(4):
                        tensor.matmul(
                            ps_s[t][32 * m : 32 * m + 32, 0:H],
                            cf16[:, 32 * g : 32 * g + 32],
                            xbb[:, 16 * t + 4 * g + m, :],
                            start=False,
                            stop=False,
                            skip_group_check=True,
                            tile_position=(0, 32 * m),
                        )
                # band B: 2 quads
                tensor.wait_ge(rsem[t], 16)
                for h in range(2):
                    for m in range(4):
                        tensor.matmul(
                            ps_s[t][32 * m : 32 * m + 32, 0:H],
                            cf16[:, 128 + 32 * h : 128 + 32 * h + 32],
                            xrr[:, roff[t] + 4 * h + m, :],
                            start=False,
                            stop=False,
                            skip_group_check=True,
                            tile_position=(0, 32 * m),
                        )
                # overflow: full-width one-hot chunks; last closes the bank
                ohcum += ovks[t]
                tensor.wait_ge(ohsem, ohcum)
                for jj in range(ovks[t]):
                    tensor.matmul(
                        ps_s[t][:, 0:H],
                        oh[:, ohcum - ovks[t] + jj, :],
                        xrr[:, roff[t] + KB2 + jj, :],
                        start=False,
                        stop=(jj == ovks[t] - 1),
                        skip_group_check=True,
                    )
                # fence: hand the bank to DVE only after writes drain
                tensor.matmul(
                    ps_x[:, 0:8], zl, zr, start=True, stop=True,
                    skip_group_check=True,
                ).then_inc(mmsem, 1)
                # transpose pooled halves (alternate banks by parity)
                tensor.wait_ge(castsem, t + 1)
                if t >= 2:
                    tensor.wait_ge(cp2sem, t - 1)
                for hb in range(2):
                    tensor.transpose(
                        ps_tab[t % 2][:, 128 * hb : 128 * (hb + 1)],
                        pool[:, t, 128 * hb : 128 * (hb + 1)],
                        ident,
                    )
                tensor.matmul(
                    ps_x[:, 0:8], zl, zr, start=True, stop=True,
                    skip_group_check=True,
                ).then_inc(trsem, 1)
                # Linear: out[s, j] = sum_h pooled_T[h, s] * W.T[h, j]
                tensor.wait_ge(cp2sem, t + 1)
                for hb in range(2):
                    tensor.matmul(
                        ps_s[t][:, 0:H],
                        sums2[:, 2 * t + hb, :],
                        cf16[:, WT_OFF + 256 * hb : WT_OFF + 256 * (hb + 1)],
                        start=(hb == 0),
                        stop=(hb == 1),
                        skip_group_check=True,
                    )
                tensor.matmul(
                    ps_x[:, 0:8], zl, zr, start=True, stop=True,
                    skip_group_check=True,
                ).then_inc(mmesem, 1)

    return nc


def kernel(x, dst_idx, dst_size, W, b):
    x = np.asarray(x)
    idx = np.asarray(dst_idx).astype(np.int64)
    W = np.asarray(W, dtype=np.float32)
    b = np.asarray(b, dtype=np.float32)
    S = int(dst_size)
    assert S == S_TOTAL and x.shape[1] == H

    counts = np.bincount(idx, minlength=S).astype(np.float32)
    inv = np.float32(1.0) / (counts + EPS)

    order = np.argsort(idx, kind="stable")
    sidx = idx[order]
    bounds = np.searchsorted(sidx, np.arange(0, S + 1, S_PER))

    x16 = x.astype(np.float16)

    bands, rests_b, ovs, ovsegs = [], [], [], []
    for i in range(N_CORES):
        lo_i, hi_i = bounds[i], bounds[i + 1]
        n_i = hi_i - lo_i
        li = (sidx[lo_i:hi_i] - S_PER * i).astype(np.int64)
        rows = order[lo_i:hi_i]
        starts = np.searchsorted(li, np.arange(S_PER + 1))
        rank = np.arange(n_i) - starts[li]
        t_, u = li // 128, li % 128
        m_, w = u // 32, u % 32
        # band A
        bm = rank < C
        cA = 16 * t_[bm] + 4 * (w[bm] // 8) + m_[bm]
        rA = 16 * (w[bm] % 8) + rank[bm]
        xband = np.zeros((128, 64, H), dtype=np.float16)
        xband[rA, cA] = x16[rows[bm]]
        bands.append(xband)
        # band B
        bm2 = (rank >= C) & (rank < C + C2)
        cB = 8 * t_[bm2] + 4 * (w[bm2] // 16) + m_[bm2]
        rB = 8 * (w[bm2] % 16) + (rank[bm2] - C)
        rests_b.append((cB, rB, rows[bm2]))
        # overflow, per tile
        om = rank >= C + C2
        ovs.append(rows[om])
        ovsegs.append((t_[om], u[om]))

    # SPMD-shared overflow chunk counts per tile
    ovks = []
    for t in range(N_TILES):
        mx = 1
        for i in range(N_CORES):
            nt = int(np.sum(ovsegs[i][0] == t))
            mx = max(mx, -(-nt // 128))
        ovks.append(mx)
    ovks = tuple(ovks)
    OVK = sum(ovks)
    NREST = 4 * KB2 + OVK
    roff = [0]
    for t in range(N_TILES):
        roff.append(roff[-1] + KB2 + ovks[t])

    key = ovks
    nc = _graph_cache.get(key)
    if nc is None:
        nc = _build(ovks)
        _graph_cache[key] = nc

    # shared f16 consts
    cf16_np = np.zeros((128, CF16_W), dtype=np.float16)
    r = np.arange(128)
    for g in range(4):  # band A stationaries
        cf16_np[r, ONES_OFF + 32 * g + 8 * g + r // C] = 1.0
    for h in range(2):  # band B stationaries
        cf16_np[r, ONES_OFF + 128 + 32 * h + 16 * h + r // C2] = 1.0
    cf16_np[r, IDENT_OFF + r] = 1.0
    cf16_np[:, IOTA_OFF : IOTA_OFF + 128] = np.arange(128, dtype=np.float16)
    for hb in range(2):
        # wt[p, 256*hb + j] = W[j, 128*hb + p]
        cf16_np[:, WT_OFF + 256 * hb : WT_OFF + 256 * (hb + 1)] = (
            W[:, 128 * hb : 128 * (hb + 1)].T.astype(np.float16)
        )

    in_maps = []
    for i in range(N_CORES):
        xr_np = np.zeros((128, NREST, H), dtype=np.float16)
        cB, rB, rowsB = rests_b[i]
        # band B chunks: tile t's chunk k lives at xrest slot roff[t] + k
        tB = cB // 8
        xr_np[rB, np.array(roff)[tB] + (cB - 8 * tB)] = x16[rowsB]
        # overflow chunks
        tv, uv = ovsegs[i]
        ovrows = ovs[i]
        cf32_np = np.zeros((128, OVIDX_OFF + OVK), dtype=np.float32)
        cf32_np[:, OVIDX_OFF:] = PAD_IDX
        for t in range(N_TILES):
            sel = tv == t
            rows_t = ovrows[sel]
            u_t = uv[sel]
            n_t = len(rows_t)
            ro = np.arange(n_t)
            xr_np[ro % 128, roff[t] + KB2 + ro // 128] = x16[rows_t]
            cf32_np[ro % 128, OVIDX_OFF + ohbase(ovks, t) + ro // 128] = u_t
        cf32_np[:, 0:4] = inv[S_PER * i : S_PER * (i + 1)].reshape(4, 128).T
        cf32_np[:, BB_OFF : BB_OFF + H] = b
        in_maps.append(
            {
                "xb": bands[i],
                "xr": xr_np,
                "cf16": cf16_np,
                "cf32": cf32_np,
            }
        )

    res = run_bass_kernel_spmd(nc, in_maps, core_ids=list(range(N_CORES)))
    return np.concatenate([res.results[i]["out"] for i in range(N_CORES)], axis=0)


def ohbase(ovks, t):
    return sum(ovks[:t])
